# revision 59
# baseline (speedup 1.0000x reference)
"""Trainium2 Bass kernel for CompressedGlobalAttention.

Problem (hardcoded shapes from the reference):
  x: (2, 8192, 1024) fp32, local_window_start=4096, 16 heads x 64 dim,
  compression ratio 8 -> 512 avg-pooled KV "pools" from the first 4096
  tokens of each batch.  out = softmax(mask(q @ k_c^T)) @ v_c projected.

Sharding (8 cores): core = b*4 + qi handles batch b and the four
512-row seq tiles g in {qi, 4+qi, 8+qi, 12+qi} ("slots" t=0..3).  The
strided assignment makes the causal-mask structure identical across
cores, so one SPMD program can statically skip fully-masked pool
chunks: slot t only computes K_SLOT[t] = [2,4,4,4] of the 4 chunks of
128 pools.  Outputs are disjoint row blocks -> no cross-core reduction.

  - pooling runs OFF the PE: the host ships x[:4096]^T pool-permuted
    and pre-scaled by 1/8 in per-engine layouts, and the pooled k/v
    source pooledT = sum_r(...) is computed by DVE windowed reduces
    (m 0-4) and Pool-engine add trees (m 5-7) in pool-half granularity,
    overlapped with the slot-0 q-projection on the PE.  kT/v for pool
    chunks 0-1 are built in phase A (all slot 0 needs); the chunk-2/3
    half is emitted inside slot 0, filling its pipeline bubbles while
    the second half of the pool data streams in.
  - score layout is transposed [pool, seq]: softmax sums come from an
    appended ones-column of v and attn@v needs no transposes.
  - a per-core pool permutation (folded into the host-side layouts)
    pins the causal boundary strip of slot 0 to pool positions
    [192,256) and of slot 1 to [448,512); slots 2,3 have no boundary.
    The strip mask becomes one core-independent additive [64,512] tile.
  - partially/fully masked pool positions are killed by per-partition
    bias columns fed to the exp() activation (-1e9 -> exp=0).
  - rows 0..7 of each batch attend to nothing (reference: uniform
    softmax over all pools); the kernel produces garbage there and the
    host overwrites them with the analytic uniform-attention value.
  - each slot's tail emits the next slot's q-projection so the PE
    chews on it while softmax chains drain; output y is shipped bf16
    (host re-adds bo).  All big inputs are host-swizzled so every DMA
    row (one SBUF partition) is one large contiguous run: DMA time is
    packet-count-bound and [128 x 8KB] beats [1024 x 1KB] by ~8x.

All matmul operands are bf16 (tolerance is 2e-2; bf16 end-to-end
measures ~6e-3): full-rate PE, half the HBM traffic.  Accumulation
stays fp32 in PSUM; softmax denominators/reciprocals stay fp32.
PSUM: sc0-2 (2 banks each) + oa (x2) = 8 banks, shared by all phases.
"""

import os
import sys

import numpy as np

NUM_HEADS = 16
HEAD_DIM = 64
RATIO = 8
B, S, D = 2, 8192, 1024
LWS = 4096
NPOOL = LWS // RATIO        # 512
SQ = S // 4                 # 2048 query rows per core
N_CORES = 8
ST = 512                    # seq tile (free dim) per slot
NST = SQ // ST              # 4 slots per core
NEG = -1.0e9
K_SLOT = (2, 4, 4, 4)       # pool chunks computed per slot
STRIP_PAIR = {0: 0, 1: 1}   # slot -> sc-pair holding the boundary strip

_RUNNER = None


def _ensure_path():
    for p in ("/opt/trn_rl_repo",):
        if p not in sys.path and os.path.isdir(p):
            sys.path.insert(0, p)


def build_program():
    """Build the Bass/Tile SPMD program (same for all 8 cores)."""
    _ensure_path()
    import concourse.bacc as bacc
    import concourse.mybir as mybir
    import concourse.tile as tile
    from contextlib import ExitStack

    f32 = mybir.dt.float32
    bf16 = mybir.dt.bfloat16
    Exp = mybir.ActivationFunctionType.Exp

    nc = bacc.Bacc("TRN2", target_bir_lowering=False, debug=False)

    # all big inputs are host-swizzled so every DMA row (one SBUF partition)
    # is one large contiguous run: DMA cost is packet-count-bound, and
    # [128 x 8KB] beats [1024 x 1KB] by ~8x in latency.
    xqt = nc.declare_dram_parameter("xqt", [128, NST * 4096], bf16, isOutput=False)
    xpT2 = nc.declare_dram_parameter("xpT2", [128, 8 * LWS], bf16, isOutput=False)
    wq = nc.declare_dram_parameter("wq", [D, D], bf16, isOutput=False)
    wk = nc.declare_dram_parameter("wk", [128, 8192], bf16, isOutput=False)
    wv = nc.declare_dram_parameter("wv", [128, 8192], bf16, isOutput=False)
    wo = nc.declare_dram_parameter("wo", [128, 8192], bf16, isOutput=False)
    bq2 = nc.declare_dram_parameter("bq2", [128, 8], f32, isOutput=False)
    bk2 = nc.declare_dram_parameter("bk2", [128, 8], f32, isOutput=False)
    bvr = nc.declare_dram_parameter("bvr", [1, D], bf16, isOutput=False)
    hs2d = nc.declare_dram_parameter("headsel2", [2, D], bf16, isOutput=False)
    dgd = nc.declare_dram_parameter("diagmask", [128, ST], bf16, isOutput=False)
    bmd = nc.declare_dram_parameter("biasmask", [128, 4], f32, isOutput=False)
    yout = nc.declare_dram_parameter("y", [SQ, D], bf16, isOutput=True)

    with tile.TileContext(nc) as tc, ExitStack() as top:
        # ---------------- persistent pools ----------------
        consts = top.enter_context(tc.tile_pool(name="consts", bufs=1))
        kTp = top.enter_context(tc.tile_pool(name="kTp", bufs=1))
        vap = top.enter_context(tc.tile_pool(name="vap", bufs=1))
        pltp = top.enter_context(tc.tile_pool(name="pltp", bufs=1))
        wqp = top.enter_context(tc.tile_pool(name="wqp", bufs=1))
        xTbp = top.enter_context(tc.tile_pool(name="xTbp", bufs=1))
        qTp = top.enter_context(tc.tile_pool(name="qTp", bufs=1))
        # phase-B PSUM pool: sc0/sc1/sc2 (2 banks each) + oa (x2) = 8 banks
        psall = top.enter_context(tc.tile_pool(name="psall", bufs=1, space="PSUM"))

        bq2_sb = consts.tile([128, 8], f32, name="bq2_sb")
        bk2_sb = consts.tile([128, 8], f32, name="bk2_sb")
        bvr_sb = consts.tile([1, D], bf16, name="bvr_sb")
        diag_sb = consts.tile([128, ST], bf16, name="diag_sb")
        bias_sb = consts.tile([128, 4], f32, name="bias_sb")
        ones1 = consts.tile([1, 128], bf16, name="ones1")
        nc.vector.memset(ones1[:], 1.0)
        # headsel2[r, c] = 1 iff (c mod 128)//64 == r ; selects the two heads
        # of a j-block for the reciprocal broadcast matmul
        headsel2 = consts.tile([2, D], bf16, name="headsel2")

        def load_small_consts():
            nc.sync.dma_start(bq2_sb[:], bq2[:, :])
            nc.sync.dma_start(bk2_sb[:], bk2[:, :])
            nc.sync.dma_start(bvr_sb[:], bvr[:, :])
            nc.sync.dma_start(diag_sb[:], dgd[:, :])
            nc.sync.dma_start(bias_sb[:], bmd[:, :])
            nc.sync.dma_start(headsel2[:], hs2d[:, :])

        # kT/pooledT are split into pool-half tiles so the chunk-0/1 half can
        # be produced (and consumed by slot 0) before chunk-2/3 data arrives
        kTh = [
            [kTp.tile([128, NPOOL // 2], bf16, name=f"kT{j}h{h}", tag=f"kT{j}h{h}")
             for h in range(2)]
            for j in range(8)
        ]
        vaug = [
            vap.tile([128, NUM_HEADS * (HEAD_DIM + 1)], bf16, name=f"vaug{i}", tag=f"vaug{i}")
            for i in range(4)
        ]
        pooledTh = [
            [pltp.tile([128, NPOOL // 2], bf16, name=f"pT{m}h{h}", tag=f"pT{m}h{h}")
             for h in range(2)]
            for m in range(8)
        ]
        wqm = [wqp.tile([128, D], bf16, name=f"wqm{m}", tag=f"wqm{m}") for m in range(8)]

        def kt_of(j, c):
            """[64|128-row, 128-col] score lhsT block for head-row r0, chunk c."""
            return kTh[j][c // 2]

        # rotating score-pair psum tags: sc0/sc1/sc2 (2 banks each) + oa (2)
        _scnt = [0]

        def next_sc():
            t = f"sc{_scnt[0] % 3}"
            _scnt[0] += 1
            return t

        def load_xTb(t):
            # two half-tiles so the first q-projection matmuls can start
            # before the whole slot's x^T has landed
            xTb = [xTbp.tile([128, 4 * ST], bf16, name=f"xTb{x}", tag=f"xTb{x}",
                             bufs=2) for x in range(2)]
            for x in range(2):
                nc.sync.dma_start(
                    xTb[x][:],
                    xqt[:, t * 4096 + x * 2048 : t * 4096 + (x + 1) * 2048],
                )
            return xTb

        def q_project(xTb):
            """j-outer q projection (wqm fully resident)."""
            qT = [qTp.tile([128, ST], bf16, name=f"qT{j}", tag=f"qT{j}", bufs=3)
                  for j in range(8)]
            for j in range(8):
                ps = psall.tile([128, ST], f32, name="qps", tag=next_sc())
                for m in range(8):
                    nc.tensor.matmul(
                        ps[:],
                        wqm[m][:, j * 128 : (j + 1) * 128],
                        xTb[m // 4][:, (m % 4) * ST : (m % 4 + 1) * ST],
                        start=(m == 0),
                        stop=(m == 7),
                    )
                nc.scalar.add(qT[j][:], ps[:], bq2_sb[:, j : j + 1])
            return qT

        # pooling reduce of half h for dim-block m (from the half-tile xph):
        #   m 0-4: layout [q(256) r(8)] -> one DVE windowed reduce
        #   m 5-7: layout [r(8) q(256)] -> Pool-engine contiguous bf16 tree
        # (both engines run in parallel, each streaming contiguous data)
        def emit_pool_reduce(m, h, xph, gsp):
            with nc.allow_low_precision(reason="pool sums; 2e-2 tol"):
                if m < 5:
                    nc.vector.tensor_reduce(
                        pooledTh[m][h][:],
                        xph[:].rearrange("p (q r) -> p q r", r=8),
                        axis=mybir.AxisListType.X,
                        op=mybir.AluOpType.add,
                    )
                else:
                    t1 = gsp.tile([128, 1024], bf16, name="t1", tag="t1", bufs=2)
                    nc.gpsimd.tensor_add(t1[:], xph[:, 0:1024], xph[:, 1024:2048])
                    t2 = gsp.tile([128, 512], bf16, name="t2", tag="t2", bufs=2)
                    nc.gpsimd.tensor_add(t2[:], t1[:, 0:512], t1[:, 512:1024])
                    nc.gpsimd.tensor_add(pooledTh[m][h][:], t2[:, 0:256], t2[:, 256:512])

        # kT[j] half h: contraction over all m-blocks of Wk^T pooledT
        def emit_kt_half(h, js=range(8), tag=None):
            for j in js:
                ps = psall.tile([128, NPOOL // 2], f32, name="ps2",
                                tag=(tag() if tag else "oa"),
                                bufs=(1 if tag else 2))
                for m in range(8):
                    nc.tensor.matmul(
                        ps[:],
                        wk_big[:, m * 1024 + j * 128 : m * 1024 + (j + 1) * 128],
                        pooledTh[m][h][:],
                        start=(m == 0),
                        stop=(m == 7),
                    )
                nc.scalar.add(kTh[j][h][:], ps[:], bk2_sb[:, j : j + 1])

        # v chunk i (128 pools): vaug[i][p, h*65+x] = (pooled @ Wv + bv | 1)
        def emit_v_chunk(i):
            ps = psall.tile([128, D], f32, name="ps3", tag=next_sc())
            for m in range(8):
                for h2 in range(2):
                    nc.tensor.matmul(
                        ps[:, h2 * 512 : (h2 + 1) * 512],
                        pooledTh[m][i // 2][:, (i % 2) * 128 : (i % 2 + 1) * 128],
                        wv_big[:, m * 1024 + h2 * 512 : m * 1024 + (h2 + 1) * 512],
                        start=(m == 0),
                        stop=False,
                    )
            for h2 in range(2):
                nc.tensor.matmul(
                    ps[:, h2 * 512 : (h2 + 1) * 512],
                    ones1[:],
                    bvr_sb[:, h2 * 512 : (h2 + 1) * 512],
                    start=False,
                    stop=True,
                )
            va = vaug[i][:].rearrange("p (h x) -> p h x", x=HEAD_DIM + 1)
            nc.vector.tensor_copy(
                va[:, :, 0:HEAD_DIM],
                ps[:].rearrange("p (h x) -> p h x", x=HEAD_DIM),
            )
            nc.vector.memset(va[:, :, HEAD_DIM : HEAD_DIM + 1], 1.0)

        # weights for k/v stay resident until the late kT half / v chunks
        # (emitted inside slot 0) are done
        wkvp = top.enter_context(tc.tile_pool(name="wkvp", bufs=1))
        wk_big = wkvp.tile([128, 8192], bf16, name="wk_big")
        wv_big = wkvp.tile([128, 8192], bf16, name="wv_big")

        # ---------------- phase A: pooled k/v (half 0) + q of slot 0 --------
        with ExitStack() as pa:
            xpp = pa.enter_context(tc.tile_pool(name="xpp", bufs=1))
            gsp = pa.enter_context(tc.tile_pool(name="gsp", bufs=1))

            # Two DMA rings in parallel (sync + scalar) for ~30% more inflow:
            #   sync:   wqm+xTb0 (q proj) -> wk -> pool half 1 (m 5,6,7,0)
            #   scalar: pool half 0 (tree tiles m 5-7 first) -> wv -> rest of
            #           pool half 1
            # Order within each ring = PE need order; the Pool-engine add
            # trees are the slowest reducer so their tiles lead the stream.
            nc.sync.dma_start(wqm[0][:], wq[0:128, :])
            xTb0 = load_xTb(0)
            for m in range(1, 8):
                nc.sync.dma_start(wqm[m][:], wq[m * 128 : (m + 1) * 128, :])
            load_small_consts()
            xph = [[xpp.tile([128, LWS // 2], bf16, name=f"xp{m}h{h}",
                             tag=f"xp{m}h{h}") for h in range(2)] for m in range(8)]
            for m in (5, 6, 7, 0, 1, 2, 3, 4):
                nc.scalar.dma_start(
                    xph[m][0][:], xpT2[:, m * LWS : m * LWS + 2048]
                )
            nc.sync.dma_start(wk_big[:], wk[:, :])
            nc.scalar.dma_start(wv_big[:], wv[:, :])
            for m in (5, 6, 7, 0):
                nc.sync.dma_start(
                    xph[m][1][:], xpT2[:, m * LWS + 2048 : m * LWS + 4096]
                )
            for m in (1, 2, 3, 4):
                nc.scalar.dma_start(
                    xph[m][1][:], xpT2[:, m * LWS + 2048 : m * LWS + 4096]
                )

            for m in range(8):
                emit_pool_reduce(m, 0, xph[m][0], gsp)

            # q projection of slot 0, emitted m-outer in two j-blocks so the
            # first matmuls only need wqm[0]+xTb0; each [128,1024] psum tile
            # holds two j-accumulators in its column halves.
            qT0 = [qTp.tile([128, ST], bf16, name=f"qT{j}", tag=f"qT{j}", bufs=3)
                   for j in range(8)]
            for jb in range(2):
                pss = [psall.tile([128, 2 * ST], f32, name="qps2", tag=next_sc())
                       for _ in range(2)]
                for m in range(8):
                    for j in range(4):
                        ph = pss[j // 2]
                        nc.tensor.matmul(
                            ph[:, (j % 2) * ST : (j % 2 + 1) * ST],
                            wqm[m][:, (4 * jb + j) * 128 : (4 * jb + j + 1) * 128],
                            xTb0[m // 4][:, (m % 4) * ST : (m % 4 + 1) * ST],
                            start=(m == 0),
                            stop=(m == 7),
                        )
                for j in range(4):
                    nc.scalar.add(qT0[4 * jb + j][:],
                                  pss[j // 2][:, (j % 2) * ST : (j % 2 + 1) * ST],
                                  bq2_sb[:, 4 * jb + j : 4 * jb + j + 1])

            emit_kt_half(0)
            emit_v_chunk(0)
            emit_v_chunk(1)
            for m in range(8):
                emit_pool_reduce(m, 1, xph[m][1], gsp)

        # ---------------- phase B: attention ----------------
        with ExitStack() as pb:
            wqop = pb.enter_context(tc.tile_pool(name="wqop", bufs=1))
            ep = pb.enter_context(tc.tile_pool(name="ep", bufs=2))
            oTp = pb.enter_context(tc.tile_pool(name="oTp", bufs=1))
            dnp = pb.enter_context(tc.tile_pool(name="dnp", bufs=1))
            ysp = pb.enter_context(tc.tile_pool(name="ysp", bufs=2))

            wo_big = wqop.tile([128, 8192], bf16, name="wo_big")
            nc.sync.dma_start(wo_big[:], wo[:, :])

            qTs = {0: qT0}
            for st in range(NST):
                s0 = st * ST
                K = K_SLOT[st]
                NPAIR = K // 2

                qT = qTs.pop(st)
                oT = [oTp.tile([128, ST], bf16, name=f"oT{j}", tag=f"oT{j}") for j in range(8)]
                strip_pair = STRIP_PAIR.get(st)
                scale = 1.0 / np.sqrt(HEAD_DIM)

                # per-head emitters, software-pipelined: scores/exp of head h
                # are emitted before attn of head h-1 so the PE never sits on
                # the exp latency; normalization runs per j-block as soon as
                # its two heads finish.
                e_of = {}
                oa_of = {}

                def emit_scores_exp(h):
                    j, r0 = h // 2, 64 * (h % 2)
                    sc = [
                        psall.tile([128, 2 * ST], f32, name=f"sc{p}", tag=next_sc())
                        for p in range(NPAIR)
                    ]
                    for c in range(K):
                        nc.tensor.matmul(
                            sc[c // 2][:, (c % 2) * ST : (c % 2 + 1) * ST],
                            kTh[j][c // 2][r0 : r0 + 64, (c % 2) * 128 : (c % 2 + 1) * 128],
                            qT[j][r0 : r0 + 64, :],
                            start=True,
                            stop=True,
                        )
                    e = [
                        ep.tile([128, 2 * ST], bf16, name=f"e{p}", tag=f"e{p}", bufs=3)
                        for p in range(NPAIR)
                    ]
                    if st == 0:
                        for c in range(2):  # bias columns 0,1
                            nc.scalar.activation(
                                e[0][:, c * ST : (c + 1) * ST],
                                sc[0][:, c * ST : (c + 1) * ST],
                                Exp, bias=bias_sb[:, c : c + 1], scale=scale,
                            )
                    elif st == 1:
                        nc.scalar.activation(e[0][:], sc[0][:], Exp, bias=0.0, scale=scale)
                        for c in range(2):  # bias columns 2,3
                            nc.scalar.activation(
                                e[1][:, c * ST : (c + 1) * ST],
                                sc[1][:, c * ST : (c + 1) * ST],
                                Exp, bias=bias_sb[:, 2 + c : 3 + c], scale=scale,
                            )
                    else:
                        for p in range(NPAIR):
                            nc.scalar.activation(e[p][:], sc[p][:], Exp, bias=0.0, scale=scale)
                    if strip_pair is not None:
                        # multiplicative 0/1 boundary mask on the strip rows
                        nc.vector.tensor_mul(
                            e[strip_pair][64:128, ST : 2 * ST],
                            e[strip_pair][64:128, ST : 2 * ST],
                            diag_sb[64:128, :],
                        )
                    e_of[h] = e

                def emit_attn(h):
                    j, r0 = h // 2, 64 * (h % 2)
                    e = e_of.pop(h)
                    oa = psall.tile([HEAD_DIM + 1, ST], f32, name="oa", tag="oa", bufs=2)
                    for c in range(K):
                        nc.tensor.matmul(
                            oa[:],
                            vaug[c][:, h * 65 : h * 65 + 65],
                            e[c // 2][:, (c % 2) * ST : (c % 2 + 1) * ST],
                            start=(c == 0),
                            stop=(c == K - 1),
                        )
                    nc.vector.tensor_copy(oT[j][r0 : r0 + 64, :], oa[0:HEAD_DIM, :])
                    oa_of[h] = oa

                denj = {}

                def emit_denrec(j):
                    # denominator gather + reciprocal for heads 2j, 2j+1
                    dj = dnp.tile([2, ST], f32, name="denj", tag="denj", bufs=3)
                    for hh in range(2):
                        oa = oa_of.pop(2 * j + hh)
                        drow = dnp.tile([1, ST], f32, name="drow", tag="drow", bufs=4)
                        nc.vector.tensor_scalar_max(
                            drow[:], oa[HEAD_DIM : HEAD_DIM + 1, :], 1e-30
                        )
                        nc.gpsimd.dma_start(dj[hh : hh + 1, :], drow[:])
                    rcf = dnp.tile([2, ST], f32, name="rcf", tag="rcf", bufs=3)
                    nc.vector.reciprocal_approx_fast(rcf[:], dj[:])
                    rcb = dnp.tile([2, ST], bf16, name="rcb", tag="rcb", bufs=3)
                    with nc.allow_low_precision(reason="softmax recip weights; 2e-2 tol"):
                        nc.vector.tensor_copy(rcb[:], rcf[:])
                    denj[j] = rcb

                def emit_norm(j):
                    rcb = denj.pop(j)
                    rps = psall.tile([128, ST], f32, name="rps", tag="oa", bufs=2)
                    nc.tensor.matmul(
                        rps[:],
                        headsel2[:, j * 128 : (j + 1) * 128],
                        rcb[:],
                        start=True,
                        stop=True,
                    )
                    nc.vector.tensor_mul(oT[j][:], oT[j][:], rps[:])

                for h in range(NUM_HEADS):
                    emit_scores_exp(h)
                    if h >= 1:
                        emit_attn(h - 1)
                        if (h - 1) % 2 == 1:
                            emit_denrec((h - 1) // 2)
                    if st == 0:
                        # the chunk-2/3 half of k/v (needed from slot 1 on) is
                        # built here, filling slot 0's pipeline bubbles after
                        # its pool data has streamed in
                        if h == 8:
                            emit_kt_half(1, js=range(0, 4), tag=next_sc)
                        elif h == 10:
                            emit_kt_half(1, js=range(4, 8), tag=next_sc)
                        elif h == 12:
                            emit_v_chunk(2)
                        elif h == 14:
                            emit_v_chunk(3)
                    if h >= 5 and h % 2 == 1:
                        emit_norm((h - 5) // 2)
                emit_attn(NUM_HEADS - 1)
                emit_denrec(NUM_HEADS // 2 - 1)

                # the next slot's q-projection is emitted before the tail
                # norms and this slot's y so the PE chews on it while the
                # softmax chains drain
                if st < NST - 1:
                    xTb = load_xTb(st + 1)
                    qTs[st + 1] = q_project(xTb)
                emit_norm(NUM_HEADS // 2 - 2)
                emit_norm(NUM_HEADS // 2 - 1)

                # final projection y[s, :] = O^T.T Wo (bo re-added on host)
                for q4 in range(4):
                    yh = psall.tile([128, D], f32, name="yh", tag=next_sc())
                    ysb = ysp.tile([128, D], bf16, name="ysb", tag="ysb")
                    for hf in range(2):
                        for j in range(8):
                            nc.tensor.matmul(
                                yh[:, hf * 512 : (hf + 1) * 512],
                                oT[j][:, q4 * 128 : (q4 + 1) * 128],
                                wo_big[:, j * 1024 + hf * 512 : j * 1024 + (hf + 1) * 512],
                                start=(j == 0),
                                stop=(j == 7),
                            )
                        with nc.allow_low_precision(reason="y output bf16; 2e-2 tol"):
                            nc.vector.tensor_copy(
                                ysb[:, hf * 512 : (hf + 1) * 512],
                                yh[:, hf * 512 : (hf + 1) * 512],
                            )
                    nc.sync.dma_start(yout[s0 + q4 * 128 : s0 + q4 * 128 + 128, :], ysb[:])

    nc.compile()
    return nc


# ---------------------------------------------------------------------------
# host side
# ---------------------------------------------------------------------------

def _bf16(a):
    import ml_dtypes

    return np.ascontiguousarray(np.asarray(a).astype(ml_dtypes.bfloat16))


def _host_constants():
    """Per-core-independent constant inputs."""
    headsel2 = np.zeros((2, D), np.float32)
    c = np.arange(D)
    headsel2[0, (c % 128) < 64] = 1.0
    headsel2[1, (c % 128) >= 64] = 1.0

    # boundary strip mask: strip row r holds the pool whose 8 source rows
    # end at local seq offset 8r+8 (same pattern for every core and slot)
    r = np.arange(64)[:, None]
    s = np.arange(ST)[None, :]
    diag = np.where(s >= 8 * r + 8, 1.0, 0.0).astype(np.float32)
    diag = np.concatenate([np.ones((64, ST), np.float32), diag], axis=0)
    return headsel2, np.ascontiguousarray(diag)


def _slot_perm(qi):
    """pos[orig_pool] = pool position after the per-core permutation.

    Places the slot-0 boundary strip (orig pools [64qi, 64qi+64)) at
    positions [192, 256) and the slot-1 strip (orig [256+64qi, +64)) at
    [448, 512), keeping everything else order-preserving.
    """
    a = 64 * qi
    pos = np.empty(NPOOL, np.int64)
    pos[0:a] = np.arange(0, a)
    pos[a : a + 64] = np.arange(192, 256)
    n1 = 192 - a
    pos[a + 64 : a + 64 + n1] = np.arange(a, 192)
    pos[a + 64 + n1 : a + 256] = np.arange(256, 256 + a)
    pos[a + 256 : a + 320] = np.arange(448, 512)
    pos[a + 320 : NPOOL] = np.arange(256 + a, 448)
    return pos


def _core_bias(qi, pos):
    """biasmask (128, 4): cols = slot0-chunk0, slot0-chunk1, slot1-chunk2,
    slot1-chunk3.  0 where the pool position is visible (or in the strip,
    handled by diag), -1e9 otherwise."""
    pool_at = np.empty(NPOOL, np.int64)
    pool_at[pos] = np.arange(NPOOL)
    bias = np.zeros((128, 4), np.float32)
    specs = [(0, 0, 0), (0, 1, 1), (1, 2, 2), (1, 3, 3)]  # (slot, chunk, col)
    for t, c, col in specs:
        g = 4 * t + qi
        s_min = 512 * g
        strip_lo, strip_hi = (192, 256) if t == 0 else (448, 512)
        for pl in range(128):
            pp = 128 * c + pl
            op = pool_at[pp]
            if strip_lo <= pp < strip_hi:
                val = 0.0
            else:
                val = 0.0 if s_min >= 8 * op + 8 else NEG
            bias[pl, col] = val
    return bias


def _numpy_reference(x, lws, Wq, bq, Wk, bk, Wv, bv, Wo, bo):
    Bx, Sx, Dx = x.shape
    H, Hd, R = NUM_HEADS, HEAD_DIM, RATIO
    if lws <= R:
        return np.zeros_like(x)
    npool = lws // R
    trunc = npool * R
    comp = x[:, :trunc, :].reshape(Bx, npool, R, Dx).mean(axis=2)
    q = (x @ Wq + bq).reshape(Bx, Sx, H, Hd).transpose(0, 2, 1, 3)
    k = (comp @ Wk + bk).reshape(Bx, npool, H, Hd).transpose(0, 2, 1, 3)
    v = (comp @ Wv + bv).reshape(Bx, npool, H, Hd).transpose(0, 2, 1, 3)
    scores = np.einsum("bhqd,bhkd->bhqk", q, k) / np.sqrt(Hd)
    mask = np.arange(Sx)[:, None] >= (np.arange(npool) + 1) * R
    scores = np.where(mask[None, None], scores, -1e9)
    scores = scores - scores.max(axis=-1, keepdims=True)
    e = np.exp(scores)
    attn = e / e.sum(axis=-1, keepdims=True)
    out = np.einsum("bhqk,bhkd->bhqd", attn, v)
    out = out.transpose(0, 2, 1, 3).reshape(Bx, Sx, H * Hd)
    return (out @ Wo + bo).astype(np.float32)


def _swizzle_w(w):
    """[1024, 1024] -> [128, 8192] with out[p, m*1024+d] = w[m*128+p, d]."""
    wb = _bf16(w)
    return np.ascontiguousarray(
        wb.reshape(8, 128, D).transpose(1, 0, 2).reshape(128, 8 * D)
    )


def make_in_maps(x, Wq, bq, Wk, bk, Wv, bv, Wo, bo):
    x = np.asarray(x, np.float32)
    xb = _bf16(x)
    headsel2, diag = _host_constants()
    headsel2 = _bf16(headsel2)
    wqb = _bf16(Wq)
    wkb, wvb, wob = _swizzle_w(Wk), _swizzle_w(Wv), _swizzle_w(Wo)
    bvrb = _bf16(np.asarray(bv, np.float32).reshape(1, D))
    bq2 = np.ascontiguousarray(np.asarray(bq, np.float32).reshape(8, 128).T)
    bk2 = np.ascontiguousarray(np.asarray(bk, np.float32).reshape(8, 128).T)
    in_maps = []
    for core in range(N_CORES):
        b, qi = core // 4, core % 4
        # xqt[p, t*4096 + m*512 + s] = x[b, 512*(4t+qi) + s, 128m + p]
        tiles = np.stack(
            [
                xb[b, 512 * (4 * t + qi) : 512 * (4 * t + qi) + 512, :]
                for t in range(NST)
            ]
        )  # [t, s, d]
        xqtc = np.ascontiguousarray(
            tiles.reshape(NST, 512, 8, 128).transpose(3, 0, 2, 1).reshape(128, NST * 4096)
        )
        pos = _slot_perm(qi)
        # pool source, two per-m layouts (see build_program):
        #   m 0-3: xpT2[p, m*4096 + pp*8 + r] (contiguous window for DVE)
        #   m 4-7: xpT2[p, m*4096 + r*512 + pp] (contiguous tree for Pool)
        xr = x[b, :LWS, :].reshape(NPOOL, RATIO, D)
        xp_pos = np.empty_like(xr)
        xp_pos[pos] = xr
        base = (xp_pos * 0.125).reshape(NPOOL, RATIO, 8, 128)  # [pp, r, m, p]
        xpc = np.empty((128, 8, LWS), np.float32)
        for m in range(5):
            # [p, (h q) r]: pp-major; halves contiguous (DVE windowed reduce)
            xpc[:, m, :] = base[:, :, m, :].transpose(2, 0, 1).reshape(128, LWS)
        for m in range(5, 8):
            # [p, h r q]: r-major within each half (Pool-engine add tree)
            bm = base[:, :, m, :].reshape(2, 256, RATIO, 128)  # [h, q, r, p]
            xpc[:, m, :] = bm.transpose(3, 0, 2, 1).reshape(128, LWS)
        xpT2c = _bf16(xpc.reshape(128, 8 * LWS))
        bias = _core_bias(qi, pos)
        in_maps.append(
            {
                "xqt": xqtc,
                "xpT2": xpT2c,
                "wq": wqb,
                "wk": wkb,
                "wv": wvb,
                "wo": wob,
                "bq2": bq2,
                "bk2": bk2,
                "bvr": bvrb,
                "headsel2": headsel2,
                "diagmask": _bf16(diag),
                "biasmask": bias,
            }
        )
    return in_maps


def assemble_output(x, Wv, bv, Wo, bo, results):
    # device output is bf16 and omits the bo bias; add it here
    y = np.empty((B, S, D), np.float32)
    for core in range(N_CORES):
        b, qi = core // 4, core % 4
        res = np.asarray(results[core]["y"], dtype=np.float32)
        for t in range(NST):
            g = 4 * t + qi
            y[b, 512 * g : 512 * g + 512, :] = res[512 * t : 512 * t + 512]
    y += np.asarray(bo, np.float32)[None, None, :]
    # rows 0..7: all pools masked -> reference uses uniform attention
    for b in range(B):
        vmean = x[b, :LWS, :].astype(np.float64).mean(axis=0).astype(np.float32)
        row = (vmean @ Wv + bv) @ Wo + bo
        y[b, 0:8, :] = row[None, :]
    return y


def kernel(**inputs):
    x = np.asarray(inputs["x"], np.float32)
    lws = int(np.asarray(inputs["local_window_start"]))
    Wq = np.asarray(inputs["Wq"], np.float32)
    bq = np.asarray(inputs["bq"], np.float32)
    Wk = np.asarray(inputs["Wk"], np.float32)
    bk = np.asarray(inputs["bk"], np.float32)
    Wv = np.asarray(inputs["Wv"], np.float32)
    bv = np.asarray(inputs["bv"], np.float32)
    Wo = np.asarray(inputs["Wo"], np.float32)
    bo = np.asarray(inputs["bo"], np.float32)

    if lws != LWS or x.shape != (B, S, D):
        return _numpy_reference(x, lws, Wq, bq, Wk, bk, Wv, bv, Wo, bo)

    try:
        _ensure_path()
        from concourse.bass_utils import run_bass_kernel_spmd

        global _RUNNER
        if _RUNNER is None:
            _RUNNER = build_program()
        nc = _RUNNER

        in_maps = make_in_maps(x, Wq, bq, Wk, bk, Wv, bv, Wo, bo)
        res = run_bass_kernel_spmd(nc, in_maps, list(range(N_CORES)))
        return assemble_output(x, Wv, bv, Wo, bo, res.results)
    except Exception as ex:  # device path unavailable -> correct host fallback
        sys.stderr.write(f"kernel: device path failed ({type(ex).__name__}: {ex}); "
                         "using host fallback\n")
        return _numpy_reference(x, lws, Wq, bq, Wk, bk, Wv, bv, Wo, bo)


if __name__ == "__main__":
    np.random.seed(0)
    xs = np.random.randn(B, S, D).astype(np.float32)
    sc = 1.0 / np.sqrt(D)
    args = dict(
        x=xs,
        local_window_start=LWS,
        Wq=np.random.randn(D, D).astype(np.float32) * sc,
        bq=np.zeros(D, np.float32),
        Wk=np.random.randn(D, D).astype(np.float32) * sc,
        bk=np.zeros(D, np.float32),
        Wv=np.random.randn(D, D).astype(np.float32) * sc,
        bv=np.zeros(D, np.float32),
        Wo=np.random.randn(D, D).astype(np.float32) * sc,
        bo=np.zeros(D, np.float32),
    )
    y = kernel(**args)
    ref = _numpy_reference(
        xs, LWS, args["Wq"], args["bq"], args["Wk"], args["bk"],
        args["Wv"], args["bv"], args["Wo"], args["bo"],
    )
    err = np.abs(y - ref)
    rel = err.max() / np.abs(ref).max()
    print("max abs err:", err.max(), "rel:", rel)
